# revision 7
# baseline (speedup 1.0000x reference)
"""Causal self-attention (RoPE) Trainium2 kernel, 8-way tensor-parallel.

Sharding (Megatron-style, zero-cost input distribution since every core
receives the full inputs): core c owns global heads {2c, 2c+1}.

Per core:
  1. qkv^T = W_slice^T @ x^T   (fp16 matmuls, fp32 psum), single pass over x^T;
     per token-chunk epilogue: bias add, RoPE on q/k (pair-swap DMA + 3 DVE
     ops), v moved to [token, d] layout via XBAR DMA transpose
  2. Per (head, batch): causal flash-style attention
       scores psum [q,k] -> +mask -> exp (ACT, fused row-sum accum) ->
       normalize by 1/l (DVE) -> XBAR DMA-transpose of P-hat -> PV matmuls
       -> y^T [d,q]
  3. AllGather of y^T slices across the 8 cores, one per (head, batch),
     so projection work can start while later attention blocks still run
  4. Output projection vs the core's 256-column slice of W_proj in two
     accumulation halves (even heads -> fp32 partial in SBUF as soon as the
     even AGs land, odd heads + partial + bias -> fp32 out [tokens, 256])

DMA traffic is batched into few large transfers (the HWDGE descriptor
generator costs ~0.6us per DMA instruction, so instruction count matters
more than bytes): weights in 2, each x chunk in 2, each projection
token-group in 1 (covering all 8 gathered head slices).

Host side shards weights, builds RoPE/mask tables, and concatenates the
8 column slices into the final [B, T, C] output.
"""

import functools
import numpy as np

import concourse.bass as bass
import concourse.mybir as mybir
import concourse.tile as tile
from concourse import bacc
from concourse.bass_utils import run_bass_kernel_spmd
from concourse.tile import add_dep_helper

F32 = mybir.dt.float32
F16 = mybir.dt.float16

N_CORES = 8
C = 2048           # model dim
H = 16             # total heads
HD = 128           # head dim
HL = 2             # heads per core
OC = C // N_CORES  # output cols per core (256)
SCALE = 1.0 / float(np.sqrt(HD))
MASK_VAL = -900.0  # additive pre-scale mask; exp arg ~ -80 -> underflows to 0


def build(B=2, T=2048, collective=True, n_cores=N_CORES):
    """Build the SPMD Bass program (identical on every core)."""
    BT = B * T
    NSTR = 3 * HL                  # qkv strips of 128 cols
    NCT = C // 128                 # contraction tiles
    NTCH = BT // 512               # token chunks for qkv
    NQC = T // 512                 # q chunks per (b, h)
    NTT = T // 128                 # token tiles per batch

    nc = bacc.Bacc(None, target_bir_lowering=False)
    xT = nc.dram_tensor("xT", [C, BT], F16, kind="ExternalInput")
    wqkv = nc.dram_tensor("wqkv", [C, NSTR * 128], F16, kind="ExternalInput")
    bqkv = nc.dram_tensor("bqkv", [128, NSTR], F32, kind="ExternalInput")
    ctil = nc.dram_tensor("ctil", [128, T], F16, kind="ExternalInput")
    stil = nc.dram_tensor("stil", [128, T], F16, kind="ExternalInput")
    wp = nc.dram_tensor("wp", [C, OC], F16, kind="ExternalInput")
    bpb = nc.dram_tensor("bpb", [128, OC], F32, kind="ExternalInput")
    cmask = nc.dram_tensor("cmask", [128, 128], F32, kind="ExternalInput")
    out = nc.dram_tensor("out", [BT, OC], F32, kind="ExternalOutput")

    with tile.TileContext(nc) as tc:
        with (
            tc.tile_pool(name="big", bufs=1) as big,
            tc.tile_pool(name="dram", bufs=1, space="DRAM") as dram,
        ):
            # ---- persistent SBUF tensors ----
            qr = big.tile([128, HL * BT], F16, tag="qr")
            kr = big.tile([128, HL * BT], F16, tag="kr")
            v_sb = big.tile([128, HL * BT], F16, tag="v_sb")
            ct_sb = big.tile([128, T], F16, tag="ct")
            st_sb = big.tile([128, T], F16, tag="st")
            cm_sb = big.tile([128, 128], F32, tag="cm")
            bq_sb = big.tile([128, NSTR], F32, tag="bq")
            bp_sb = big.tile([128, OC], F32, tag="bp")

            # DRAM bounce buffers: one AllGather per (local head j, batch b)
            # every (head, batch) gather is split into two column halves so each
            # projection half starts while the second half is still on the wire
            split_ag = T >= 1024
            nh = 2 if split_ag else 1
            hw_ = T // nh
            agin = {}
            agout = {}
            for j in range(HL):
                for b in range(B):
                    for h in range(nh):
                        agin[(j, b, h)] = dram.tile([128, hw_], F16,
                                                    name=f"agin{j}_{b}_{h}")
                        agout[(j, b, h)] = dram.tile([n_cores * 128, hw_], F16,
                                                     name=f"agout{j}_{b}_{h}")

            # ================= Phase A: QKV + RoPE + v-transpose =================
            with (
                tc.tile_pool(name="wq", bufs=1) as wq_pool,
                tc.tile_pool(name="xt", bufs=2) as xt_pool,
                tc.tile_pool(name="rope", bufs=2) as rope_pool,
                tc.tile_pool(name="stage", bufs=2) as stage_pool,
                tc.tile_pool(name="qkv_ps", bufs=4, space="PSUM") as qkv_ps,
            ):
                # one merged load per 1024-row half: [128, 8, NSTR*128]
                w_all = wq_pool.tile([128, NCT * NSTR * 128], F16, tag="w_all",
                                     name="w_all")
                WS = NSTR * 128  # 768 cols per contraction tile
                for hh in range(2):
                    nc.sync.dma_start(
                        w_all[:, hh * 8 * WS:(hh + 1) * 8 * WS]
                        .rearrange("p (a c) -> p a c", c=WS),
                        wqkv[hh * 1024:(hh + 1) * 1024, :]
                        .rearrange("(a p) c -> p a c", p=128))
                nc.sync.dma_start(bq_sb[:], bqkv[:, :])

                def load_x_chunk(tch):
                    xt_t = xt_pool.tile([128, NCT * 512], F16, tag="xt",
                                        name="xt")
                    for hh in range(2):
                        nc.sync.dma_start(
                            xt_t[:, hh * 8 * 512:(hh + 1) * 8 * 512]
                            .rearrange("p (a c) -> p a c", c=512),
                            xT[hh * 1024:(hh + 1) * 1024,
                               tch * 512:(tch + 1) * 512]
                            .rearrange("(a p) c -> p a c", p=128))
                    return xt_t

                xts_first = load_x_chunk(0)
                for tch in range(NTCH):
                    if tch == min(1, NTCH - 1):
                        # constants land after the first xT burst is in flight
                        nc.sync.dma_start(ct_sb[:], ctil[:, :])
                        nc.sync.dma_start(st_sb[:], stil[:, :])
                        nc.sync.dma_start(cm_sb[:], cmask[:, :])
                        nc.sync.dma_start(bp_sb[:], bpb[:, :])
                    tw = (tch * 512) % T        # token offset within batch
                    tok = slice(tw, tw + 512)
                    xt_t = xts_first if tch == 0 else load_x_chunk(tch)
                    stg = stage_pool.tile([128, NSTR * 512], F16, tag="stg",
                                          name="stg")
                    for s in range(NSTR):
                        ps = qkv_ps.tile([128, 512], F32, name="qkvps")
                        for ctn in range(NCT):
                            nc.tensor.matmul(
                                ps[:],
                                w_all[:, ctn * WS + s * 128:
                                      ctn * WS + (s + 1) * 128],
                                xt_t[:, ctn * 512:(ctn + 1) * 512],
                                start=(ctn == 0), stop=(ctn == NCT - 1))
                        nc.scalar.activation(
                            stg[:, s * 512:(s + 1) * 512], ps[:],
                            mybir.ActivationFunctionType.Identity,
                            bias=bq_sb[:, s:s + 1], scale=1.0)
                    # pair-swap of the 4 q/k strips in two strided DMAs
                    # (on the ACT hwdge queue, right behind the epilogues that
                    # produce stg, so they never block SP-queue x prefetches)
                    sw = rope_pool.tile([128, 4 * 512], F16, tag="sw", name="sw")
                    nc.scalar.dma_start(sw[0:127:2, :], stg[1:128:2, 0:2048])
                    nc.scalar.dma_start(sw[1:128:2, :], stg[0:127:2, 0:2048])
                    for j in range(HL):
                        for si, dst in ((j, qr), (2 + j, kr)):
                            dstsl = dst[:, j * BT + tch * 512:
                                        j * BT + (tch + 1) * 512]
                            tmp = rope_pool.tile([128, 512], F16, tag=f"rt{si}",
                                                 name=f"rt{si}")
                            nc.vector.tensor_mul(
                                dstsl, stg[:, si * 512:(si + 1) * 512],
                                ct_sb[:, tok])
                            nc.vector.tensor_mul(
                                tmp[:], sw[:, si * 512:(si + 1) * 512],
                                st_sb[:, tok])
                            nc.vector.tensor_add(dstsl, dstsl, tmp[:])
                        # v -> [token, d] layout via XBAR transpose
                        nc.scalar.dma_start_transpose(
                            v_sb[:, j * BT + tch * 512:
                                 j * BT + (tch + 1) * 512]
                            .rearrange("p (a c) -> p a c", c=128),
                            stg[:, (4 + j) * 512:(5 + j) * 512])

            # ============ Phase B: attention + AG + interleaved projection ============
            with (
                tc.tile_pool(name="p", bufs=6) as p_pool,
                tc.tile_pool(name="pt", bufs=2) as pt_pool,
                tc.tile_pool(name="stat", bufs=24) as stat_pool,
                tc.tile_pool(name="yts", bufs=4) as yts_pool,
                tc.tile_pool(name="wpp", bufs=1) as wp_pool,
                tc.tile_pool(name="ygs", bufs=3) as ygs_pool,
                tc.tile_pool(name="part", bufs=1) as part_pool,
                tc.tile_pool(name="ot", bufs=4) as ot_pool,
                tc.tile_pool(name="sc_ps", bufs=4, space="PSUM") as sc_ps,
                tc.tile_pool(name="y_ps", bufs=2, space="PSUM") as y_ps,
                tc.tile_pool(name="o_ps", bufs=1, space="PSUM") as o_ps,
            ):
                wpall = wp_pool.tile([128, H * OC], F16, tag="wpall",
                                     name="wpall")
                nc.sync.dma_start(
                    wpall[:].rearrange("p (a c) -> p a c", c=OC),
                    wp[:, :].rearrange("(a p) c -> p a c", p=128))
                partial = {}
                cc_insts = {}
                agin_dmas = {}

                def attention_block(j, b):
                    base = j * BT + b * T
                    for qc in range(NQC):
                        nkt = 4 * qc + 4
                        # P-hat^T: per-qt contiguous regions of stride 2048;
                        # region qtw holds [k (part|tile), q(128)] tile-blocks
                        ptall = pt_pool.tile([128, 4 * 2048], F16, tag="ptall",
                                             name="ptall")
                        for qtw in range(4):
                            qt = qc * 4 + qtw
                            kext = (qt + 1) * 128
                            qtile = qr[:, base + qt * 128: base + (qt + 1) * 128]
                            ptile = p_pool.tile([128, T], F16, tag="P", name="P")
                            lparts = []
                            off = 0
                            while off < kext:
                                n = min(512, kext - off)
                                ps = sc_ps.tile([128, 512], F32, name="scps")
                                nc.tensor.matmul(
                                    ps[:, :n], qtile, kr[:, base + off: base + off + n],
                                    start=True, stop=True)
                                if off + n == kext:
                                    nc.vector.tensor_add(
                                        ps[:, n - 128:n], ps[:, n - 128:n], cm_sb[:])
                                lp = stat_pool.tile([128, 1], F32, tag="lp", name="lp")
                                nc.scalar.activation(
                                    ptile[:, off:off + n], ps[:, :n],
                                    mybir.ActivationFunctionType.Exp,
                                    scale=SCALE, accum_out=lp[:])
                                lparts.append(lp)
                                off += n
                            lsum = stat_pool.tile([128, 1], F32, tag="ls", name="ls")
                            if len(lparts) == 1:
                                lsum = lparts[0]
                            else:
                                nc.vector.tensor_add(lsum[:], lparts[0][:], lparts[1][:])
                                for lp in lparts[2:]:
                                    nc.vector.tensor_add(lsum[:], lsum[:], lp[:])
                            rec = stat_pool.tile([128, 1], F32, tag="rec", name="rec")
                            nc.vector.reciprocal(rec[:], lsum[:])
                            nc.vector.tensor_scalar_mul(
                                ptile[:, :kext], ptile[:, :kext], rec[:])
                            nc.sync.dma_start_transpose(
                                ptall[:, qtw * 2048: qtw * 2048 + kext]
                                .rearrange("p (a c) -> p a c", c=128),
                                ptile[:, 0:kext])
                        ptr = ptall[:].rearrange("p (a c) -> p a c", c=2048)
                        psy = y_ps.tile([128, 512], F32, name="psy")
                        for kt in range(nkt):
                            q0 = max(0, (kt - 4 * qc))
                            nc.tensor.matmul(
                                psy[:, q0 * 128:512],
                                v_sb[:, base + kt * 128: base + (kt + 1) * 128],
                                ptr[:, q0:4, kt * 128:(kt + 1) * 128],
                                start=(kt == 0), stop=(kt == nkt - 1))
                        yt = yts_pool.tile([128, 512], F16, tag="yt", name="yt")
                        nc.vector.tensor_copy(yt[:], psy[:])
                        hqc = max(1, NQC // nh)
                        dstb = agin[(j, b, qc // hqc)]
                        dst_ap = dstb[:, (qc % hqc) * 512:(qc % hqc + 1) * 512]
                        d = nc.gpsimd.dma_start(dst_ap, yt[:])
                        agin_dmas.setdefault((j, b), []).append(d)
                    ccs = []
                    for h in range(nh):
                        if collective:
                            cc = nc.gpsimd.collective_compute(
                                "AllGather",
                                mybir.AluOpType.bypass,
                                replica_groups=[list(range(n_cores))],
                                ins=[agin[(j, b, h)].opt()],
                                outs=[agout[(j, b, h)].opt()],
                            )
                        else:
                            cc = nc.sync.dma_start(agout[(j, b, h)][0:128, :],
                                                   agin[(j, b, h)][:, :])
                        cci = cc.ins if hasattr(cc, "ins") else cc
                        ndep = len(agin_dmas[(j, b)]) // nh
                        for d in agin_dmas[(j, b)][h * ndep:(h + 1) * ndep]:
                            di = d.ins if hasattr(d, "ins") else d
                            add_dep_helper(cci, di,
                                           reason="collective reads agin after y")
                        ccs.append(cci)
                    cc_insts[(j, b)] = ccs

                def proj_half(b, par, first):
                    """Accumulate heads of parity `par` for batch b.

                    first=True: psum + bias -> fp32 partial tiles in SBUF.
                    first=False: psum + partial -> output DMA.
                    """
                    for tg0 in range(0, NTT, 4):
                        h = (tg0 * 128) // hw_
                        c0 = tg0 * 128 - h * hw_
                        # one DMA: all 8 gathered [128,512] head slices
                        ygall = ygs_pool.tile([128, n_cores * 512], F16,
                                              tag="ygall", name="ygall")
                        d = nc.gpsimd.dma_start(
                            ygall[:].rearrange("p (a c) -> p a c", c=512),
                            agout[(par, b, h)][:, c0:c0 + 512]
                            .rearrange("(a p) c -> p a c", p=128))
                        di = d.ins if hasattr(d, "ins") else d
                        add_dep_helper(di, cc_insts[(par, b)][h],
                                       reason="proj reads agout after collective")
                        for p0 in range(0, 4, 2):
                            pss = [o_ps.tile([128, OC], F32, tag=f"op{i}",
                                             name=f"op{i}")
                                   for i in range(2)]
                            for ci in range(n_cores):
                                g = 2 * ci + par
                                for i in range(2):
                                    nc.tensor.matmul(
                                        pss[i][:],
                                        ygall[:, ci * 512 + (p0 + i) * 128:
                                              ci * 512 + (p0 + i + 1) * 128],
                                        wpall[:, g * OC:(g + 1) * OC],
                                        start=(ci == 0), stop=(ci == n_cores - 1))
                            for i in range(2):
                                tt = tg0 + p0 + i
                                if first:
                                    pt_t = part_pool.tile([128, OC], F32,
                                                          tag=f"part{b}_{tt}",
                                                          name=f"part{b}_{tt}")
                                    nc.vector.tensor_add(pt_t[:], pss[i][:], bp_sb[:])
                                    partial[(b, tt)] = pt_t
                                else:
                                    ot = ot_pool.tile([128, OC], F32, tag="ot",
                                                      name="ot")
                                    nc.vector.tensor_add(
                                        ot[:], pss[i][:], partial[(b, tt)][:])
                                    r0 = b * T + tt * 128
                                    nc.gpsimd.dma_start(out[r0:r0 + 128, :], ot[:])

                for b in range(B):
                    attention_block(0, b)
                for b in range(B):
                    proj_half(b, 0, first=True)
                for b in range(B):
                    attention_block(1, b)
                    proj_half(b, 1, first=False)
    nc.compile()
    return nc


def _prep_inputs(x, W_attn, b_attn, W_proj, b_proj, cos, sin, core, B, T):
    """Host-side shard prep for one core."""
    BT = B * T
    xT = np.ascontiguousarray(x.reshape(BT, C).T).astype(np.float16)

    cols = []
    bvals = []
    for part in range(3):  # q, k, v
        for j in range(HL):
            h = 2 * core + j
            sl = slice(part * C + h * HD, part * C + (h + 1) * HD)
            cols.append(W_attn[:, sl])
            bvals.append(b_attn[sl])
    wqkv = np.concatenate(cols, axis=1).astype(np.float16)
    bqkv = np.ascontiguousarray(
        np.concatenate(bvals).astype(np.float32).reshape(3 * HL, 128).T)

    # RoPE tables: ctil[p, t] = cos[t, p//2]; stil[2i] = -sin, stil[2i+1] = +sin
    cosr = np.repeat(cos.T, 2, axis=0)  # [128, T]
    sinr = np.repeat(sin.T, 2, axis=0)
    sgn = np.where((np.arange(128) % 2) == 0, -1.0, 1.0)[:, None]
    ctil = cosr.astype(np.float16)
    stil = (sinr * sgn).astype(np.float16)

    wp_c = W_proj[:, core * OC:(core + 1) * OC].astype(np.float16)
    bpb = np.tile(b_proj[core * OC:(core + 1) * OC].astype(np.float32), (128, 1))
    ii, jj = np.mgrid[0:128, 0:128]
    cmask = np.where(jj <= ii, 0.0, MASK_VAL).astype(np.float32)
    return {
        "xT": xT, "wqkv": wqkv, "bqkv": bqkv, "ctil": ctil, "stil": stil,
        "wp": wp_c, "bpb": bpb, "cmask": cmask,
    }


@functools.lru_cache(maxsize=2)
def _built(B, T):
    return build(B=B, T=T)


_warmed = set()


def kernel(x, W_attn, b_attn, W_proj, b_proj, cos, sin):
    x = np.asarray(x, dtype=np.float32)
    W_attn = np.asarray(W_attn, dtype=np.float32)
    b_attn = np.asarray(b_attn, dtype=np.float32)
    W_proj = np.asarray(W_proj, dtype=np.float32)
    b_proj = np.asarray(b_proj, dtype=np.float32)
    cos = np.asarray(cos, dtype=np.float32)
    sin = np.asarray(sin, dtype=np.float32)

    B, T, Cv = x.shape
    assert Cv == C
    nc = _built(B, T)
    in_maps = [_prep_inputs(x, W_attn, b_attn, W_proj, b_proj, cos, sin, c, B, T)
               for c in range(N_CORES)]
    if (B, T) not in _warmed:
        # The very first execution of a freshly loaded NEFF has been observed
        # to deliver stale/uninitialized collective buffers; run once and
        # discard, then run for real.
        run_bass_kernel_spmd(nc, in_maps, core_ids=list(range(N_CORES)))
        _warmed.add((B, T))
    res = run_bass_kernel_spmd(nc, in_maps, core_ids=list(range(N_CORES)))
    outs = [res.results[c]["out"] for c in range(N_CORES)]
    full = np.concatenate(outs, axis=1)  # [BT, C]
    return full.reshape(B, T, C).astype(np.float32)


# revision 13
# speedup vs baseline: 1.3194x; 1.3194x over previous
"""Causal self-attention (RoPE) Trainium2 kernel, 8-way tensor-parallel.

Sharding (Megatron-style, zero-cost input distribution since every core
receives the full inputs): core c owns global heads {2c, 2c+1}.

Per core:
  1. qkv^T = W_slice^T @ x^T   (fp16 matmuls, fp32 psum), single pass over x^T;
     per token-chunk epilogue: bias add, RoPE on q/k (pair-swap DMA + 3 DVE
     ops), v moved to [token, d] layout via XBAR DMA transpose
  2. Per (head, batch): causal flash-style attention
       scores psum [q,k] -> +mask -> exp (ACT, fused row-sum accum) ->
       normalize by 1/l (DVE) -> XBAR DMA-transpose of P-hat -> PV matmuls
       -> y^T [d,q]
  3. AllGather of y^T slices across the 8 cores, one per (head, batch),
     so projection work can start while later attention blocks still run
  4. Output projection vs the core's 256-column slice of W_proj in two
     accumulation halves (even heads -> fp32 partial in SBUF as soon as the
     even AGs land, odd heads + partial + bias -> fp32 out [tokens, 256])

DMA traffic is batched into few large transfers (the HWDGE descriptor
generator costs ~0.6us per DMA instruction, so instruction count matters
more than bytes): weights in 2, each x chunk in 2, each projection
token-group in 1 (covering all 8 gathered head slices).

Host side shards weights, builds RoPE/mask tables, and concatenates the
8 column slices into the final [B, T, C] output.
"""

import functools
import numpy as np

import concourse.bass as bass
import concourse.mybir as mybir
import concourse.tile as tile
from concourse import bacc
from concourse.bass_utils import run_bass_kernel_spmd
from concourse.masks import make_identity
from concourse.tile import add_dep_helper

F32 = mybir.dt.float32
F16 = mybir.dt.float16

N_CORES = 8
C = 2048           # model dim
H = 16             # total heads
HD = 128           # head dim
HL = 2             # heads per core
OC = C // N_CORES  # output cols per core (256)
SCALE = 1.0 / float(np.sqrt(HD))
MASK_VAL = -900.0  # additive pre-scale mask; exp arg ~ -80 -> underflows to 0


def build(B=2, T=2048, collective=True, n_cores=N_CORES):
    """Build the SPMD Bass program (identical on every core)."""
    BT = B * T
    NSTR = 3 * HL                  # qkv strips of 128 cols
    NCT = C // 128                 # contraction tiles
    NTCH = BT // 512               # token chunks for qkv
    NQC = T // 512                 # q chunks per (b, h)
    NTT = T // 128                 # token tiles per batch

    nc = bacc.Bacc(None, target_bir_lowering=False)
    xT = nc.dram_tensor("xT", [C, BT], F16, kind="ExternalInput")
    wqkv = nc.dram_tensor("wqkv", [C, NSTR * 128], F16, kind="ExternalInput")
    bqkv = nc.dram_tensor("bqkv", [128, NSTR], F32, kind="ExternalInput")
    ctil = nc.dram_tensor("ctil", [128, T], F16, kind="ExternalInput")
    stil = nc.dram_tensor("stil", [128, T], F16, kind="ExternalInput")
    wp = nc.dram_tensor("wp", [C, OC], F16, kind="ExternalInput")
    bpb = nc.dram_tensor("bpb", [128, OC], F32, kind="ExternalInput")
    cmask = nc.dram_tensor("cmask", [128, 128], F32, kind="ExternalInput")
    out = nc.dram_tensor("out", [BT, OC], F32, kind="ExternalOutput")

    with tile.TileContext(nc) as tc:
        with (
            tc.tile_pool(name="big", bufs=1) as big,
            tc.tile_pool(name="dram", bufs=1, space="DRAM") as dram,
        ):
            # ---- persistent SBUF tensors ----
            qr = big.tile([128, HL * BT], F16, tag="qr")
            kr = big.tile([128, HL * BT], F16, tag="kr")
            v_sb = big.tile([128, HL * BT], F16, tag="v_sb")
            ct_sb = big.tile([128, T], F16, tag="ct")
            st_sb = big.tile([128, T], F16, tag="st")
            ident = big.tile([128, 128], F16, tag="ident")
            cm_sb = big.tile([128, 128], F32, tag="cm")
            bq_sb = big.tile([128, NSTR], F32, tag="bq")
            bp_sb = big.tile([128, OC], F32, tag="bp")

            # DRAM bounce buffers: one AllGather per (local head j, batch b)
            # every (head, batch) gather is split into two column halves so each
            # projection half starts while the second half is still on the wire
            split_ag = T >= 1024
            nh = 2 if split_ag else 1
            hw_ = T // nh
            agin = {}
            agout = {}
            for j in range(HL):
                for b in range(B):
                    for h in range(nh):
                        agin[(j, b, h)] = dram.tile([128, hw_], F16,
                                                    name=f"agin{j}_{b}_{h}")
                        agout[(j, b, h)] = dram.tile([n_cores * 128, hw_], F16,
                                                     name=f"agout{j}_{b}_{h}")

            # ================= Phase A: QKV + RoPE + v-transpose =================
            with (
                tc.tile_pool(name="wq", bufs=1) as wq_pool,
                tc.tile_pool(name="xt", bufs=2) as xt_pool,
                tc.tile_pool(name="rope", bufs=2) as rope_pool,
                tc.tile_pool(name="stage", bufs=2) as stage_pool,
                tc.tile_pool(name="qkv_ps", bufs=4, space="PSUM") as qkv_ps,
            ):
                # one merged load per 1024-row half: [128, 8, NSTR*128]
                w_all = wq_pool.tile([128, NCT * NSTR * 128], F16, tag="w_all",
                                     name="w_all")
                WS = NSTR * 128  # 768 cols per contraction tile
                for hh in range(2):
                    nc.sync.dma_start(
                        w_all[:, hh * 8 * WS:(hh + 1) * 8 * WS]
                        .rearrange("p (a c) -> p a c", c=WS),
                        wqkv[hh * 1024:(hh + 1) * 1024, :]
                        .rearrange("(a p) c -> p a c", p=128))
                nc.sync.dma_start(bq_sb[:], bqkv[:, :])
                make_identity(nc, ident[:])

                def load_x_chunk(tch):
                    xt_t = xt_pool.tile([128, NCT * 512], F16, tag="xt",
                                        name="xt")
                    for hh in range(2):
                        nc.sync.dma_start(
                            xt_t[:, hh * 8 * 512:(hh + 1) * 8 * 512]
                            .rearrange("p (a c) -> p a c", c=512),
                            xT[hh * 1024:(hh + 1) * 1024,
                               tch * 512:(tch + 1) * 512]
                            .rearrange("(a p) c -> p a c", p=128))
                    return xt_t

                xts_first = load_x_chunk(0)
                for tch in range(NTCH):
                    if tch == min(1, NTCH - 1):
                        # constants land after the first xT burst is in flight
                        nc.sync.dma_start(ct_sb[:], ctil[:, :])
                        nc.sync.dma_start(st_sb[:], stil[:, :])
                        nc.sync.dma_start(cm_sb[:], cmask[:, :])
                        nc.sync.dma_start(bp_sb[:], bpb[:, :])
                    tw = (tch * 512) % T        # token offset within batch
                    tok = slice(tw, tw + 512)
                    xt_t = xts_first if tch == 0 else load_x_chunk(tch)
                    stg = stage_pool.tile([128, NSTR * 512], F16, tag="stg",
                                          name="stg")
                    for s in range(NSTR):
                        ps = qkv_ps.tile([128, 512], F32, name="qkvps")
                        for ctn in range(NCT):
                            nc.tensor.matmul(
                                ps[:],
                                w_all[:, ctn * WS + s * 128:
                                      ctn * WS + (s + 1) * 128],
                                xt_t[:, ctn * 512:(ctn + 1) * 512],
                                start=(ctn == 0), stop=(ctn == NCT - 1))
                        nc.scalar.activation(
                            stg[:, s * 512:(s + 1) * 512], ps[:],
                            mybir.ActivationFunctionType.Identity,
                            bias=bq_sb[:, s:s + 1], scale=1.0)
                    # pair-swap of the 4 q/k strips in two strided DMAs
                    # (on the ACT hwdge queue, right behind the epilogues that
                    # produce stg, so they never block SP-queue x prefetches)
                    sw = rope_pool.tile([128, 4 * 512], F16, tag="sw", name="sw")
                    nc.scalar.dma_start(sw[0:127:2, :], stg[1:128:2, 0:2048])
                    nc.scalar.dma_start(sw[1:128:2, :], stg[0:127:2, 0:2048])
                    for j in range(HL):
                        for si, dst in ((j, qr), (2 + j, kr)):
                            dstsl = dst[:, j * BT + tch * 512:
                                        j * BT + (tch + 1) * 512]
                            tmp = rope_pool.tile([128, 512], F16, tag=f"rt{si}",
                                                 name=f"rt{si}")
                            nc.vector.tensor_mul(
                                dstsl, stg[:, si * 512:(si + 1) * 512],
                                ct_sb[:, tok])
                            nc.vector.tensor_mul(
                                tmp[:], sw[:, si * 512:(si + 1) * 512],
                                st_sb[:, tok])
                            nc.vector.tensor_add(dstsl, dstsl, tmp[:])
                        # v -> [token, d] layout via XBAR transpose
                        nc.scalar.dma_start_transpose(
                            v_sb[:, j * BT + tch * 512:
                                 j * BT + (tch + 1) * 512]
                            .rearrange("p (a c) -> p a c", c=128),
                            stg[:, (4 + j) * 512:(5 + j) * 512])

            # ============ Phase B: attention + AG + interleaved projection ============
            with (
                tc.tile_pool(name="p", bufs=4) as p_pool,
                tc.tile_pool(name="pt", bufs=3) as pt_pool,
                tc.tile_pool(name="stat", bufs=16) as stat_pool,
                tc.tile_pool(name="yts", bufs=4) as yts_pool,
                tc.tile_pool(name="wpp", bufs=1) as wp_pool,
                tc.tile_pool(name="ygs", bufs=3) as ygs_pool,
                tc.tile_pool(name="part", bufs=1) as part_pool,
                tc.tile_pool(name="ot", bufs=4) as ot_pool,
                tc.tile_pool(name="sc_ps", bufs=2, space="PSUM") as sc_ps,
                tc.tile_pool(name="tp_ps", bufs=2, space="PSUM") as tp_ps,
                tc.tile_pool(name="y_ps", bufs=2, space="PSUM") as y_ps,
                tc.tile_pool(name="o_ps", bufs=1, space="PSUM") as o_ps,
            ):
                wpall = wp_pool.tile([128, H * OC], F16, tag="wpall",
                                     name="wpall")
                nc.sync.dma_start(
                    wpall[:].rearrange("p (a c) -> p a c", c=OC),
                    wp[:, :].rearrange("(a p) c -> p a c", p=128))
                partial = {}
                cc_insts = {}
                agin_dmas = {}

                def attention_block(j, b):
                    base = j * BT + b * T
                    for qc in range(NQC):
                        nkt = 4 * qc + 4
                        ptall = pt_pool.tile([128, 8192], F16, tag="ptall",
                                             name="ptall")
                        for qtw in range(4):
                            qt = qc * 4 + qtw
                            kext = (qt + 1) * 128
                            qtile = qr[:, base + qt * 128: base + (qt + 1) * 128]
                            ptile = p_pool.tile([128, T], F16, tag="P", name="P")
                            lparts = []
                            off = 0
                            while off < kext:
                                n = min(512, kext - off)
                                ps = sc_ps.tile([128, 512], F32, name="scps")
                                nc.tensor.matmul(
                                    ps[:, :n], qtile, kr[:, base + off: base + off + n],
                                    start=True, stop=True)
                                if off + n == kext:
                                    nc.vector.tensor_add(
                                        ps[:, n - 128:n], ps[:, n - 128:n], cm_sb[:])
                                lp = stat_pool.tile([128, 1], F32, tag="lp", name="lp")
                                nc.scalar.activation(
                                    ptile[:, off:off + n], ps[:, :n],
                                    mybir.ActivationFunctionType.Exp,
                                    scale=SCALE, accum_out=lp[:])
                                lparts.append(lp)
                                off += n
                            lsum = stat_pool.tile([128, 1], F32, tag="ls", name="ls")
                            if len(lparts) == 1:
                                lsum = lparts[0]
                            else:
                                nc.vector.tensor_add(lsum[:], lparts[0][:], lparts[1][:])
                                for lp in lparts[2:]:
                                    nc.vector.tensor_add(lsum[:], lsum[:], lp[:])
                            rec = stat_pool.tile([128, 1], F32, tag="rec", name="rec")
                            nc.vector.reciprocal(rec[:], lsum[:])
                            nc.vector.tensor_scalar_mul(
                                ptile[:, :kext], ptile[:, :kext], rec[:])
                            kt = 0
                            while kt <= qt:
                                if kt + 1 <= qt:
                                    tp = tp_ps.tile([128, 256], F16, tag="tp", name="tp")
                                    nc.tensor.transpose(
                                        tp[:, 0:128],
                                        ptile[:, kt * 128:(kt + 1) * 128], ident[:])
                                    nc.tensor.transpose(
                                        tp[:, 128:256],
                                        ptile[:, (kt + 1) * 128:(kt + 2) * 128],
                                        ident[:])
                                    dst = ptall[:].rearrange(
                                        "p (a b) -> p a b", b=512)[
                                        :, kt:kt + 2, qtw * 128:(qtw + 1) * 128]
                                    nc.vector.tensor_copy(
                                        dst,
                                        tp[:].rearrange("p (a b) -> p a b", a=2))
                                    kt += 2
                                else:
                                    tp = tp_ps.tile([128, 256], F16, tag="tp", name="tp")
                                    nc.tensor.transpose(
                                        tp[:, 0:128],
                                        ptile[:, kt * 128:(kt + 1) * 128], ident[:])
                                    nc.vector.tensor_copy(
                                        ptall[:, kt * 512 + qtw * 128:
                                              kt * 512 + (qtw + 1) * 128],
                                        tp[:, 0:128])
                                    kt += 1
                        psy = y_ps.tile([128, 512], F32, name="psy")
                        for kt in range(nkt):
                            qstart = max(0, (kt - 4 * qc)) * 128
                            nc.tensor.matmul(
                                psy[:, qstart:512],
                                v_sb[:, base + kt * 128: base + (kt + 1) * 128],
                                ptall[:, kt * 512 + qstart: kt * 512 + 512],
                                start=(kt == 0), stop=(kt == nkt - 1))
                        yt = yts_pool.tile([128, 512], F16, tag="yt", name="yt")
                        nc.vector.tensor_copy(yt[:], psy[:])
                        hqc = max(1, NQC // nh)
                        dstb = agin[(j, b, qc // hqc)]
                        dst_ap = dstb[:, (qc % hqc) * 512:(qc % hqc + 1) * 512]
                        d = nc.gpsimd.dma_start(dst_ap, yt[:])
                        agin_dmas.setdefault((j, b), []).append(d)
                    ccs = []
                    for h in range(nh):
                        if collective:
                            cc = nc.gpsimd.collective_compute(
                                "AllGather",
                                mybir.AluOpType.bypass,
                                replica_groups=[list(range(n_cores))],
                                ins=[agin[(j, b, h)].opt()],
                                outs=[agout[(j, b, h)].opt()],
                            )
                        else:
                            cc = nc.sync.dma_start(agout[(j, b, h)][0:128, :],
                                                   agin[(j, b, h)][:, :])
                        cci = cc.ins if hasattr(cc, "ins") else cc
                        ndep = len(agin_dmas[(j, b)]) // nh
                        for d in agin_dmas[(j, b)][h * ndep:(h + 1) * ndep]:
                            di = d.ins if hasattr(d, "ins") else d
                            add_dep_helper(cci, di,
                                           reason="collective reads agin after y")
                        ccs.append(cci)
                    cc_insts[(j, b)] = ccs

                def proj_half(b, par, first):
                    """Accumulate heads of parity `par` for batch b.

                    first=True: psum + bias -> fp32 partial tiles in SBUF.
                    first=False: psum + partial -> output DMA.
                    """
                    for tg0 in range(0, NTT, 4):
                        h = (tg0 * 128) // hw_
                        c0 = tg0 * 128 - h * hw_
                        # one DMA: all 8 gathered [128,512] head slices
                        ygall = ygs_pool.tile([128, n_cores * 512], F16,
                                              tag="ygall", name="ygall")
                        d = nc.gpsimd.dma_start(
                            ygall[:].rearrange("p (a c) -> p a c", c=512),
                            agout[(par, b, h)][:, c0:c0 + 512]
                            .rearrange("(a p) c -> p a c", p=128))
                        di = d.ins if hasattr(d, "ins") else d
                        add_dep_helper(di, cc_insts[(par, b)][h],
                                       reason="proj reads agout after collective")
                        for p0 in range(0, 4, 2):
                            pss = [o_ps.tile([128, OC], F32, tag=f"op{i}",
                                             name=f"op{i}")
                                   for i in range(2)]
                            for ci in range(n_cores):
                                g = 2 * ci + par
                                for i in range(2):
                                    nc.tensor.matmul(
                                        pss[i][:],
                                        ygall[:, ci * 512 + (p0 + i) * 128:
                                              ci * 512 + (p0 + i + 1) * 128],
                                        wpall[:, g * OC:(g + 1) * OC],
                                        start=(ci == 0), stop=(ci == n_cores - 1))
                            for i in range(2):
                                tt = tg0 + p0 + i
                                if first:
                                    pt_t = part_pool.tile([128, OC], F32,
                                                          tag=f"part{b}_{tt}",
                                                          name=f"part{b}_{tt}")
                                    nc.vector.tensor_add(pt_t[:], pss[i][:], bp_sb[:])
                                    partial[(b, tt)] = pt_t
                                else:
                                    ot = ot_pool.tile([128, OC], F32, tag="ot",
                                                      name="ot")
                                    nc.vector.tensor_add(
                                        ot[:], pss[i][:], partial[(b, tt)][:])
                                    r0 = b * T + tt * 128
                                    nc.gpsimd.dma_start(out[r0:r0 + 128, :], ot[:])

                for b in range(B):
                    attention_block(0, b)
                for b in range(B):
                    proj_half(b, 0, first=True)
                for b in range(B):
                    attention_block(1, b)
                    proj_half(b, 1, first=False)
    nc.compile()
    return nc


def _prep_inputs(x, W_attn, b_attn, W_proj, b_proj, cos, sin, core, B, T):
    """Host-side shard prep for one core."""
    BT = B * T
    xT = np.ascontiguousarray(x.reshape(BT, C).T).astype(np.float16)

    cols = []
    bvals = []
    for part in range(3):  # q, k, v
        for j in range(HL):
            h = 2 * core + j
            sl = slice(part * C + h * HD, part * C + (h + 1) * HD)
            cols.append(W_attn[:, sl])
            bvals.append(b_attn[sl])
    wqkv = np.concatenate(cols, axis=1).astype(np.float16)
    bqkv = np.ascontiguousarray(
        np.concatenate(bvals).astype(np.float32).reshape(3 * HL, 128).T)

    # RoPE tables: ctil[p, t] = cos[t, p//2]; stil[2i] = -sin, stil[2i+1] = +sin
    cosr = np.repeat(cos.T, 2, axis=0)  # [128, T]
    sinr = np.repeat(sin.T, 2, axis=0)
    sgn = np.where((np.arange(128) % 2) == 0, -1.0, 1.0)[:, None]
    ctil = cosr.astype(np.float16)
    stil = (sinr * sgn).astype(np.float16)

    wp_c = W_proj[:, core * OC:(core + 1) * OC].astype(np.float16)
    bpb = np.tile(b_proj[core * OC:(core + 1) * OC].astype(np.float32), (128, 1))
    ii, jj = np.mgrid[0:128, 0:128]
    cmask = np.where(jj <= ii, 0.0, MASK_VAL).astype(np.float32)
    return {
        "xT": xT, "wqkv": wqkv, "bqkv": bqkv, "ctil": ctil, "stil": stil,
        "wp": wp_c, "bpb": bpb, "cmask": cmask,
    }


@functools.lru_cache(maxsize=2)
def _built(B, T):
    return build(B=B, T=T)


_warmed = set()


def kernel(x, W_attn, b_attn, W_proj, b_proj, cos, sin):
    x = np.asarray(x, dtype=np.float32)
    W_attn = np.asarray(W_attn, dtype=np.float32)
    b_attn = np.asarray(b_attn, dtype=np.float32)
    W_proj = np.asarray(W_proj, dtype=np.float32)
    b_proj = np.asarray(b_proj, dtype=np.float32)
    cos = np.asarray(cos, dtype=np.float32)
    sin = np.asarray(sin, dtype=np.float32)

    B, T, Cv = x.shape
    assert Cv == C
    nc = _built(B, T)
    in_maps = [_prep_inputs(x, W_attn, b_attn, W_proj, b_proj, cos, sin, c, B, T)
               for c in range(N_CORES)]
    if (B, T) not in _warmed:
        # The very first execution of a freshly loaded NEFF has been observed
        # to deliver stale/uninitialized collective buffers; run once and
        # discard, then run for real.
        run_bass_kernel_spmd(nc, in_maps, core_ids=list(range(N_CORES)))
        _warmed.add((B, T))
    res = run_bass_kernel_spmd(nc, in_maps, core_ids=list(range(N_CORES)))
    outs = [res.results[c]["out"] for c in range(N_CORES)]
    full = np.concatenate(outs, axis=1)  # [BT, C]
    return full.reshape(B, T, C).astype(np.float32)


# revision 18
# speedup vs baseline: 1.3563x; 1.0280x over previous
"""Causal self-attention (RoPE) Trainium2 kernel, 8-way tensor-parallel.

Sharding (Megatron-style, zero-cost input distribution since every core
receives the full inputs): core c owns global heads {2c, 2c+1}.

Per core:
  1. qkv^T = W_slice^T @ x^T   (fp16 matmuls, fp32 psum), single pass over x^T;
     per token-chunk epilogue: bias add, RoPE on q/k (pair-swap DMA + 3 DVE
     ops), v moved to [token, d] layout via XBAR DMA transpose
  2. Per (head, batch): causal flash-style attention
       scores psum [q,k] -> +mask -> exp (ACT, fused row-sum accum) ->
       normalize by 1/l (DVE) -> XBAR DMA-transpose of P-hat -> PV matmuls
       -> y^T [d,q]
  3. AllGather of y^T slices across the 8 cores, one per (head, batch),
     so projection work can start while later attention blocks still run
  4. Output projection vs the core's 256-column slice of W_proj in two
     accumulation halves (even heads -> fp32 partial in SBUF as soon as the
     even AGs land, odd heads + partial + bias -> fp32 out [tokens, 256])

DMA traffic is batched into few large transfers (the HWDGE descriptor
generator costs ~0.6us per DMA instruction, so instruction count matters
more than bytes): weights in 2, each x chunk in 2, each projection
token-group in 1 (covering all 8 gathered head slices).

Host side shards weights, builds RoPE/mask tables, and concatenates the
8 column slices into the final [B, T, C] output.
"""

import functools
import numpy as np

import concourse.bass as bass
import concourse.mybir as mybir
import concourse.tile as tile
from concourse import bacc
from concourse.bass_utils import run_bass_kernel_spmd
from concourse.masks import make_identity
from concourse.tile import add_dep_helper

F32 = mybir.dt.float32
F16 = mybir.dt.float16

N_CORES = 8
C = 2048           # model dim
H = 16             # total heads
HD = 128           # head dim
HL = 2             # heads per core
OC = C // N_CORES  # output cols per core (256)
SCALE = 1.0 / float(np.sqrt(HD))
MASK_VAL = -900.0  # additive pre-scale mask; exp arg ~ -80 -> underflows to 0


def build(B=2, T=2048, collective=True, n_cores=N_CORES):
    """Build the SPMD Bass program (identical on every core)."""
    BT = B * T
    NSTR = 3 * HL                  # qkv strips of 128 cols
    NCT = C // 128                 # contraction tiles
    NTCH = BT // 512               # token chunks for qkv
    NQC = T // 512                 # q chunks per (b, h)
    NTT = T // 128                 # token tiles per batch

    nc = bacc.Bacc(None, target_bir_lowering=False)
    xT = nc.dram_tensor("xT", [C, BT], F16, kind="ExternalInput")
    wqkv = nc.dram_tensor("wqkv", [C, NSTR * 128], F16, kind="ExternalInput")
    bqkv = nc.dram_tensor("bqkv", [128, NSTR], F32, kind="ExternalInput")
    ctil = nc.dram_tensor("ctil", [128, T], F16, kind="ExternalInput")
    stil = nc.dram_tensor("stil", [128, T], F16, kind="ExternalInput")
    wp = nc.dram_tensor("wp", [C, OC], F16, kind="ExternalInput")
    bpb = nc.dram_tensor("bpb", [128, OC], F32, kind="ExternalInput")
    cmask = nc.dram_tensor("cmask", [128, 128], F32, kind="ExternalInput")
    out = nc.dram_tensor("out", [BT, OC], F32, kind="ExternalOutput")

    with tile.TileContext(nc) as tc:
        with (
            tc.tile_pool(name="big", bufs=1) as big,
            tc.tile_pool(name="dram", bufs=1, space="DRAM") as dram,
        ):
            # ---- persistent SBUF tensors ----
            qr = big.tile([128, HL * BT], F16, tag="qr")
            kr = big.tile([128, HL * BT], F16, tag="kr")
            v_sb = big.tile([128, HL * BT], F16, tag="v_sb")
            ct_sb = big.tile([128, T], F16, tag="ct")
            st_sb = big.tile([128, T], F16, tag="st")
            ident = big.tile([128, 128], F16, tag="ident")
            cm_sb = big.tile([128, 128], F32, tag="cm")
            bq_sb = big.tile([128, NSTR], F32, tag="bq")
            bp_sb = big.tile([128, OC], F32, tag="bp")

            # DRAM bounce buffers: one AllGather per (local head j, batch b)
            # every (head, batch) gather is split into two column halves so each
            # projection half starts while the second half is still on the wire
            split_ag = T >= 1024
            nh = 2 if split_ag else 1
            hw_ = T // nh
            agin = {}
            agout = {}
            for j in range(HL):
                for b in range(B):
                    for h in range(nh):
                        agin[(j, b, h)] = dram.tile([128, hw_], F16,
                                                    name=f"agin{j}_{b}_{h}")
                        agout[(j, b, h)] = dram.tile([n_cores * 128, hw_], F16,
                                                     name=f"agout{j}_{b}_{h}")

            # ================= Phase A: QKV + RoPE + v-transpose =================
            with (
                tc.tile_pool(name="wq", bufs=1) as wq_pool,
                tc.tile_pool(name="xt", bufs=2) as xt_pool,
                tc.tile_pool(name="rope", bufs=2) as rope_pool,
                tc.tile_pool(name="stage", bufs=2) as stage_pool,
                tc.tile_pool(name="qkv_ps", bufs=6, space="PSUM") as qkv_ps,
            ):
                w_all = wq_pool.tile([128, NCT * NSTR * 128], F16, tag="w_all",
                                     name="w_all")
                WS = NSTR * 128  # 768 cols per contraction tile

                def load_w_part(hh, nparts=2):
                    rows = NCT // nparts  # contraction tiles per part
                    nc.sync.dma_start(
                        w_all[:, hh * rows * WS:(hh + 1) * rows * WS]
                        .rearrange("p (a c) -> p a c", c=WS),
                        wqkv[hh * rows * 128:(hh + 1) * rows * 128, :]
                        .rearrange("(a p) c -> p a c", p=128))

                def load_x_chunk(tch, xt_t=None, nparts=2, part=None):
                    if xt_t is None:
                        xt_t = xt_pool.tile([128, NCT * 512], F16, tag="xt",
                                            name="xt")
                    rows = NCT // nparts
                    for hh in range(nparts) if part is None else [part]:
                        nc.sync.dma_start(
                            xt_t[:, hh * rows * 512:(hh + 1) * rows * 512]
                            .rearrange("p (a c) -> p a c", c=512),
                            xT[hh * rows * 128:(hh + 1) * rows * 128,
                               tch * 512:(tch + 1) * 512]
                            .rearrange("(a p) c -> p a c", p=128))
                    return xt_t

                # first chunk: interleave quarter-granular w and x loads so the
                # leading matmuls are gated on ~1.2 MB, not 5 MB
                xts_first = xt_pool.tile([128, NCT * 512], F16, tag="xt",
                                         name="xt")
                for qq in range(4):
                    load_w_part(qq, nparts=4)
                    load_x_chunk(0, xts_first, nparts=4, part=qq)
                nc.sync.dma_start(bq_sb[:], bqkv[:, :])
                make_identity(nc, ident[:])
                for tch in range(NTCH):
                    if tch == min(1, NTCH - 1):
                        # constants land after the first xT burst is in flight
                        nc.sync.dma_start(ct_sb[:], ctil[:, :])
                        nc.sync.dma_start(st_sb[:], stil[:, :])
                        nc.sync.dma_start(cm_sb[:], cmask[:, :])
                        nc.sync.dma_start(bp_sb[:], bpb[:, :])
                    tw = (tch * 512) % T        # token offset within batch
                    tok = slice(tw, tw + 512)
                    xt_t = xts_first if tch == 0 else load_x_chunk(tch)
                    stg = stage_pool.tile([128, NSTR * 512], F16, tag="stg",
                                          name="stg")
                    for s in range(NSTR):
                        ps = qkv_ps.tile([128, 512], F32, name="qkvps")
                        for ctn in range(NCT):
                            nc.tensor.matmul(
                                ps[:],
                                w_all[:, ctn * WS + s * 128:
                                      ctn * WS + (s + 1) * 128],
                                xt_t[:, ctn * 512:(ctn + 1) * 512],
                                start=(ctn == 0), stop=(ctn == NCT - 1))
                        nc.scalar.activation(
                            stg[:, s * 512:(s + 1) * 512], ps[:],
                            mybir.ActivationFunctionType.Identity,
                            bias=bq_sb[:, s:s + 1], scale=1.0)
                    # pair-swap of the 4 q/k strips in two strided DMAs
                    # (on the ACT hwdge queue, right behind the epilogues that
                    # produce stg, so they never block SP-queue x prefetches)
                    sw = rope_pool.tile([128, 4 * 512], F16, tag="sw", name="sw")
                    nc.scalar.dma_start(sw[0:127:2, :], stg[1:128:2, 0:2048])
                    nc.scalar.dma_start(sw[1:128:2, :], stg[0:127:2, 0:2048])
                    for j in range(HL):
                        for si, dst in ((j, qr), (2 + j, kr)):
                            dstsl = dst[:, j * BT + tch * 512:
                                        j * BT + (tch + 1) * 512]
                            tmp = rope_pool.tile([128, 512], F16, tag=f"rt{si}",
                                                 name=f"rt{si}")
                            nc.vector.tensor_mul(
                                dstsl, stg[:, si * 512:(si + 1) * 512],
                                ct_sb[:, tok])
                            nc.vector.tensor_mul(
                                tmp[:], sw[:, si * 512:(si + 1) * 512],
                                st_sb[:, tok])
                            nc.vector.tensor_add(dstsl, dstsl, tmp[:])
                        # v -> [token, d] layout via XBAR transpose
                        nc.scalar.dma_start_transpose(
                            v_sb[:, j * BT + tch * 512:
                                 j * BT + (tch + 1) * 512]
                            .rearrange("p (a c) -> p a c", c=128),
                            stg[:, (4 + j) * 512:(5 + j) * 512])

            # ============ Phase B: attention + AG + interleaved projection ============
            with (
                tc.tile_pool(name="p", bufs=6) as p_pool,
                tc.tile_pool(name="pt", bufs=3) as pt_pool,
                tc.tile_pool(name="stat", bufs=24) as stat_pool,
                tc.tile_pool(name="yts", bufs=4) as yts_pool,
                tc.tile_pool(name="wpp", bufs=1) as wp_pool,
                tc.tile_pool(name="ygs", bufs=3) as ygs_pool,
                tc.tile_pool(name="part", bufs=1) as part_pool,
                tc.tile_pool(name="ot", bufs=4) as ot_pool,
                tc.tile_pool(name="sc_ps", bufs=2, space="PSUM") as sc_ps,
                tc.tile_pool(name="tp_ps", bufs=2, space="PSUM") as tp_ps,
                tc.tile_pool(name="y_ps", bufs=2, space="PSUM") as y_ps,
                tc.tile_pool(name="o_ps", bufs=1, space="PSUM") as o_ps,
            ):
                wpall = wp_pool.tile([128, H * OC], F16, tag="wpall",
                                     name="wpall")
                nc.sync.dma_start(
                    wpall[:].rearrange("p (a c) -> p a c", c=OC),
                    wp[:, :].rearrange("(a p) c -> p a c", p=128))
                partial = {}
                cc_insts = {}
                agin_dmas = {}

                def attention_block(j, b):
                    base = j * BT + b * T
                    for qc in range(NQC):
                        nkt = 4 * qc + 4
                        ptall = pt_pool.tile([128, 8192], F16, tag="ptall",
                                             name="ptall")
                        for qtw in range(4):
                            qt = qc * 4 + qtw
                            kext = (qt + 1) * 128
                            qtile = qr[:, base + qt * 128: base + (qt + 1) * 128]
                            ptile = p_pool.tile([128, T], F16, tag="P", name="P")
                            lparts = []
                            off = 0
                            while off < kext:
                                n = min(512, kext - off)
                                ps = sc_ps.tile([128, 512], F32, name="scps")
                                nc.tensor.matmul(
                                    ps[:, :n], qtile, kr[:, base + off: base + off + n],
                                    start=True, stop=True)
                                if off + n == kext:
                                    nc.vector.tensor_add(
                                        ps[:, n - 128:n], ps[:, n - 128:n], cm_sb[:])
                                lp = stat_pool.tile([128, 1], F32, tag="lp", name="lp")
                                nc.scalar.activation(
                                    ptile[:, off:off + n], ps[:, :n],
                                    mybir.ActivationFunctionType.Exp,
                                    scale=SCALE, accum_out=lp[:])
                                lparts.append(lp)
                                off += n
                            lsum = stat_pool.tile([128, 1], F32, tag="ls", name="ls")
                            if len(lparts) == 1:
                                lsum = lparts[0]
                            else:
                                nc.vector.tensor_add(lsum[:], lparts[0][:], lparts[1][:])
                                for lp in lparts[2:]:
                                    nc.vector.tensor_add(lsum[:], lsum[:], lp[:])
                            rec = stat_pool.tile([128, 1], F32, tag="rec", name="rec")
                            nc.vector.reciprocal(rec[:], lsum[:])
                            nc.vector.tensor_scalar_mul(
                                ptile[:, :kext], ptile[:, :kext], rec[:])
                            kt = 0
                            while kt <= qt:
                                if kt + 1 <= qt:
                                    tp = tp_ps.tile([128, 256], F16, tag="tp", name="tp")
                                    nc.tensor.transpose(
                                        tp[:, 0:128],
                                        ptile[:, kt * 128:(kt + 1) * 128], ident[:])
                                    nc.tensor.transpose(
                                        tp[:, 128:256],
                                        ptile[:, (kt + 1) * 128:(kt + 2) * 128],
                                        ident[:])
                                    dst = ptall[:].rearrange(
                                        "p (a b) -> p a b", b=512)[
                                        :, kt:kt + 2, qtw * 128:(qtw + 1) * 128]
                                    nc.vector.tensor_copy(
                                        dst,
                                        tp[:].rearrange("p (a b) -> p a b", a=2))
                                    kt += 2
                                else:
                                    tp = tp_ps.tile([128, 256], F16, tag="tp", name="tp")
                                    nc.tensor.transpose(
                                        tp[:, 0:128],
                                        ptile[:, kt * 128:(kt + 1) * 128], ident[:])
                                    nc.vector.tensor_copy(
                                        ptall[:, kt * 512 + qtw * 128:
                                              kt * 512 + (qtw + 1) * 128],
                                        tp[:, 0:128])
                                    kt += 1
                        psy = y_ps.tile([128, 512], F32, name="psy")
                        for kt in range(nkt):
                            qstart = max(0, (kt - 4 * qc)) * 128
                            nc.tensor.matmul(
                                psy[:, qstart:512],
                                v_sb[:, base + kt * 128: base + (kt + 1) * 128],
                                ptall[:, kt * 512 + qstart: kt * 512 + 512],
                                start=(kt == 0), stop=(kt == nkt - 1))
                        yt = yts_pool.tile([128, 512], F16, tag="yt", name="yt")
                        nc.vector.tensor_copy(yt[:], psy[:])
                        hqc = max(1, NQC // nh)
                        dstb = agin[(j, b, qc // hqc)]
                        dst_ap = dstb[:, (qc % hqc) * 512:(qc % hqc + 1) * 512]
                        d = nc.gpsimd.dma_start(dst_ap, yt[:])
                        agin_dmas.setdefault((j, b), []).append(d)
                    ccs = []
                    for h in range(nh):
                        if collective:
                            cc = nc.gpsimd.collective_compute(
                                "AllGather",
                                mybir.AluOpType.bypass,
                                replica_groups=[list(range(n_cores))],
                                ins=[agin[(j, b, h)].opt()],
                                outs=[agout[(j, b, h)].opt()],
                            )
                        else:
                            cc = nc.sync.dma_start(agout[(j, b, h)][0:128, :],
                                                   agin[(j, b, h)][:, :])
                        cci = cc.ins if hasattr(cc, "ins") else cc
                        ndep = len(agin_dmas[(j, b)]) // nh
                        for d in agin_dmas[(j, b)][h * ndep:(h + 1) * ndep]:
                            di = d.ins if hasattr(d, "ins") else d
                            add_dep_helper(cci, di,
                                           reason="collective reads agin after y")
                        ccs.append(cci)
                    cc_insts[(j, b)] = ccs

                def proj_half(b, par, first):
                    """Accumulate heads of parity `par` for batch b.

                    first=True: psum + bias -> fp32 partial tiles in SBUF.
                    first=False: psum + partial -> output DMA.
                    """
                    for tg0 in range(0, NTT, 4):
                        h = (tg0 * 128) // hw_
                        c0 = tg0 * 128 - h * hw_
                        # one DMA: all 8 gathered [128,512] head slices
                        ygall = ygs_pool.tile([128, n_cores * 512], F16,
                                              tag="ygall", name="ygall")
                        d = nc.gpsimd.dma_start(
                            ygall[:].rearrange("p (a c) -> p a c", c=512),
                            agout[(par, b, h)][:, c0:c0 + 512]
                            .rearrange("(a p) c -> p a c", p=128))
                        di = d.ins if hasattr(d, "ins") else d
                        add_dep_helper(di, cc_insts[(par, b)][h],
                                       reason="proj reads agout after collective")
                        for p0 in range(0, 4, 2):
                            pss = [o_ps.tile([128, OC], F32, tag=f"op{i}",
                                             name=f"op{i}")
                                   for i in range(2)]
                            for ci in range(n_cores):
                                g = 2 * ci + par
                                for i in range(2):
                                    nc.tensor.matmul(
                                        pss[i][:],
                                        ygall[:, ci * 512 + (p0 + i) * 128:
                                              ci * 512 + (p0 + i + 1) * 128],
                                        wpall[:, g * OC:(g + 1) * OC],
                                        start=(ci == 0), stop=(ci == n_cores - 1))
                            for i in range(2):
                                tt = tg0 + p0 + i
                                if first:
                                    pt_t = part_pool.tile([128, OC], F32,
                                                          tag=f"part{b}_{tt}",
                                                          name=f"part{b}_{tt}")
                                    nc.vector.tensor_add(pt_t[:], pss[i][:], bp_sb[:])
                                    partial[(b, tt)] = pt_t
                                else:
                                    ot = ot_pool.tile([128, OC], F32, tag="ot",
                                                      name="ot")
                                    nc.vector.tensor_add(
                                        ot[:], pss[i][:], partial[(b, tt)][:])
                                    r0 = b * T + tt * 128
                                    nc.gpsimd.dma_start(out[r0:r0 + 128, :], ot[:])

                for b in range(B):
                    attention_block(0, b)
                for b in range(B):
                    proj_half(b, 0, first=True)
                for b in range(B):
                    attention_block(1, b)
                    proj_half(b, 1, first=False)
    nc.compile()
    return nc


def _prep_inputs(x, W_attn, b_attn, W_proj, b_proj, cos, sin, core, B, T):
    """Host-side shard prep for one core."""
    BT = B * T
    xT = np.ascontiguousarray(x.reshape(BT, C).T).astype(np.float16)

    cols = []
    bvals = []
    for part in range(3):  # q, k, v
        for j in range(HL):
            h = 2 * core + j
            sl = slice(part * C + h * HD, part * C + (h + 1) * HD)
            cols.append(W_attn[:, sl])
            bvals.append(b_attn[sl])
    wqkv = np.concatenate(cols, axis=1).astype(np.float16)
    bqkv = np.ascontiguousarray(
        np.concatenate(bvals).astype(np.float32).reshape(3 * HL, 128).T)

    # RoPE tables: ctil[p, t] = cos[t, p//2]; stil[2i] = -sin, stil[2i+1] = +sin
    cosr = np.repeat(cos.T, 2, axis=0)  # [128, T]
    sinr = np.repeat(sin.T, 2, axis=0)
    sgn = np.where((np.arange(128) % 2) == 0, -1.0, 1.0)[:, None]
    ctil = cosr.astype(np.float16)
    stil = (sinr * sgn).astype(np.float16)

    wp_c = W_proj[:, core * OC:(core + 1) * OC].astype(np.float16)
    bpb = np.tile(b_proj[core * OC:(core + 1) * OC].astype(np.float32), (128, 1))
    ii, jj = np.mgrid[0:128, 0:128]
    cmask = np.where(jj <= ii, 0.0, MASK_VAL).astype(np.float32)
    return {
        "xT": xT, "wqkv": wqkv, "bqkv": bqkv, "ctil": ctil, "stil": stil,
        "wp": wp_c, "bpb": bpb, "cmask": cmask,
    }


@functools.lru_cache(maxsize=2)
def _built(B, T):
    return build(B=B, T=T)


_warmed = set()


def kernel(x, W_attn, b_attn, W_proj, b_proj, cos, sin):
    x = np.asarray(x, dtype=np.float32)
    W_attn = np.asarray(W_attn, dtype=np.float32)
    b_attn = np.asarray(b_attn, dtype=np.float32)
    W_proj = np.asarray(W_proj, dtype=np.float32)
    b_proj = np.asarray(b_proj, dtype=np.float32)
    cos = np.asarray(cos, dtype=np.float32)
    sin = np.asarray(sin, dtype=np.float32)

    B, T, Cv = x.shape
    assert Cv == C
    nc = _built(B, T)
    in_maps = [_prep_inputs(x, W_attn, b_attn, W_proj, b_proj, cos, sin, c, B, T)
               for c in range(N_CORES)]
    if (B, T) not in _warmed:
        # The very first execution of a freshly loaded NEFF has been observed
        # to deliver stale/uninitialized collective buffers; run once and
        # discard, then run for real.
        run_bass_kernel_spmd(nc, in_maps, core_ids=list(range(N_CORES)))
        _warmed.add((B, T))
    res = run_bass_kernel_spmd(nc, in_maps, core_ids=list(range(N_CORES)))
    outs = [res.results[c]["out"] for c in range(N_CORES)]
    full = np.concatenate(outs, axis=1)  # [BT, C]
    return full.reshape(B, T, C).astype(np.float32)
